# revision 30
# baseline (speedup 1.0000x reference)
"""DotGATHead Trainium2 kernel: LN -> 4-head masked attention -> MLP (2x swish+LN+linear) -> LN.

Sharding: query rows. 8 cores = 4 batches x 2 query-halves. Each core gets its
batch's x (rolled so its query rows are always rows 0..1023 -> one SPMD program),
computes K/V over all 2048 nodes, attention + MLP for its 1024 queries.

Design (final):
- T layout throughout: features/keys on partitions, nodes/queries on free dim.
- All matmul operands fp16 (FWL weight loads, 1 cyc/row transposes); psum fp32.
  fp16 keeps the PE p-state ramped (2.4GHz needs ~3us continuous execution).
- Mask applied as a -30 additive bias injected into the score PSUM via an
  identity matmul (exp then underflows masked lanes to exact fp16 zero) --
  removes all DVE mask multiplies from the exp->attnV critical path.
- P3 is chunk-major (c outer, heads inner) so the per-chunk MLP (P4) overlaps
  attention of the other chunk. exp per ktile-pair; fp16 adder tree + ones
  matmul for softmax denominators; no max-subtraction (LN-bounded scores).
- MLP: LN1/LN2 standardization folded into fc1/fc2 via K=1 augmentation
  matmuls (stats via replicated-ones matmuls in T layout); fc2 in ROW
  orientation so the final LN needs no transpose; LN2's rstd never applied
  (final LN is invariant to per-row positive scales).
- DMA: weights+fc on the gpsimd queue ahead of everything; x in 4-tile
  batches then the mask on sync; psum->SBUF copies split across ACT/DVE.
"""

import numpy as np
import ml_dtypes

import concourse.bass as bass
import concourse.mybir as mybir
import concourse.tile as tile
from concourse import bacc
from concourse.bass_utils import run_bass_kernel_spmd
from concourse.masks import make_identity

B, A, D, HEADS, HD = 4, 2048, 512, 4, 128
P = 128
QLOC = 1024              # query rows per core
NQS = QLOC // P          # 8 query subtiles
NKT = A // P             # 16 key tiles
NEO = D // P             # 4 feature partition-tiles
NQC = QLOC // 512        # 2 query chunks of 512
NAC = A // 512           # 4 node chunks of 512
EPS = 1e-5
F32 = mybir.dt.float32
F16 = mybir.dt.float16
AF = mybir.ActivationFunctionType
OP = mybir.AluOpType

_compiled = {}


def _build(use_v_bias, use_qk_bias, use_final_affine):
    nc = bacc.Bacc("TRN2", target_bir_lowering=False, debug=False, num_devices=8)

    x_d = nc.dram_tensor("x", [A, D], F32, kind="ExternalInput")
    mask_d = nc.dram_tensor("maskT", [P, NKT, QLOC], F16, kind="ExternalInput")
    wq_d = nc.dram_tensor("wq_t", [D, D], F16, kind="ExternalInput")
    wk_d = nc.dram_tensor("wk_t", [D, D], F16, kind="ExternalInput")
    wv_d = nc.dram_tensor("wv_t", [D, D], F16, kind="ExternalInput")
    fc1_d = nc.dram_tensor("fc1_t", [D, D], F16, kind="ExternalInput")
    fc2_d = nc.dram_tensor("fc2_t", [D, D], F16, kind="ExternalInput")
    fc1b_d = nc.dram_tensor("fc1b", [P, NEO], F32, kind="ExternalInput")
    fc2sum_d = nc.dram_tensor("fc2sum", [1, D], F16, kind="ExternalInput")
    fc2b_d = nc.dram_tensor("fc2b", [1, D], F16, kind="ExternalInput")
    fc1sum_d = nc.dram_tensor("fc1sum", [1, D], F16, kind="ExternalInput")
    fc1brow_d = nc.dram_tensor("fc1brow", [1, D], F16, kind="ExternalInput")
    qkvb_d = nc.dram_tensor("qkvb", [P, 3 * NEO], F32, kind="ExternalInput")
    naff_d = nc.dram_tensor("naff", [1, 2 * D], F32, kind="ExternalInput")
    vbrow_d = nc.dram_tensor("vbrow", [1, D], F32, kind="ExternalInput")
    y_d = nc.dram_tensor("y", [QLOC, D], F32, kind="ExternalOutput")

    with tile.TileContext(nc) as tc:
        with tc.tile_pool(name="const", bufs=1) as const, \
             tc.tile_pool(name="orow", bufs=1) as orow:
            # ---- constants / small tiles ----
            ident = const.tile([P, P], F16)
            make_identity(nc, ident[:])
            eps_t = const.tile([P, 1], F32)
            nc.vector.memset(eps_t[:], EPS)
            ones_inv = const.tile([P, P], F16)       # 1/D for stat matmuls
            nc.vector.memset(ones_inv[:], 1.0 / D)
            ones_h = const.tile([P, P], F16)
            nc.vector.memset(ones_h[:], 1.0)
            fc1b = const.tile([P, NEO], F32)
            nc.gpsimd.dma_start(out=fc1b[:], in_=fc1b_d[:])
            fc2sum = const.tile([1, D], F16)
            nc.gpsimd.dma_start(out=fc2sum[:], in_=fc2sum_d[:])
            fc2b = const.tile([1, D], F16)
            nc.gpsimd.dma_start(out=fc2b[:], in_=fc2b_d[:])
            fc1sum = const.tile([1, D], F16)
            nc.gpsimd.dma_start(out=fc1sum[:], in_=fc1sum_d[:])
            fc1brow = const.tile([1, D], F16)
            nc.gpsimd.dma_start(out=fc1brow[:], in_=fc1brow_d[:])
            qkvb = const.tile([P, 3 * NEO], F32)
            if use_qk_bias or use_v_bias:
                nc.gpsimd.dma_start(out=qkvb[:], in_=qkvb_d[:])
            vb_rep = const.tile([P, D], F32)
            if use_v_bias:
                vb_ap = vbrow_d[:, :]
                nc.gpsimd.dma_start(out=vb_rep[:], in_=bass.AP(
                    tensor=vb_ap.tensor, offset=vb_ap.offset,
                    ap=[[0, P], [1, D]]))
            naff = const.tile([P, 2 * D], F32)
            if use_final_affine:
                naff_ap = naff_d[:, :]
                nc.gpsimd.dma_start(out=naff[:], in_=bass.AP(
                    tensor=naff_ap.tensor, offset=naff_ap.offset,
                    ap=[[0, P], [1, 2 * D]]))

            outT = orow.tile([P, NEO, QLOC], F16)   # attention output, T layout [d, h, q]
            fc1 = orow.tile([P, NEO, D], F16)
            fc2 = orow.tile([P, NEO, D], F16)

            # ======== attention working set (closed before late P4) ========
            with tc.tile_pool(name="attw", bufs=1) as attw:
                KT = attw.tile([P, HEADS, A], F16)       # K^T/sqrt(hd): [d, h, node]
                QT = attw.tile([P, HEADS, QLOC], F16)    # Q^T: [d, h, q]
                Vg = attw.tile([P, NKT, D], F16)         # V rows [node, f]
                maskT = attw.tile([P, NKT, QLOC], F16)

                # ---- P1: LN(x) row tiles + transpose -> xnT; P2: projections ----
                with tc.tile_pool(name="xnp", bufs=1) as xnp, \
                     tc.tile_pool(name="p1t", bufs=6) as p1t, \
                     tc.tile_pool(name="p1ps", bufs=2, space="PSUM") as p1ps, \
                     tc.tile_pool(name="p2ps", bufs=3, space="PSUM") as p2ps:
                    xnT = xnp.tile([P, NEO, A], F16)
                    wq = xnp.tile([P, NEO, D], F16)
                    wk = xnp.tile([P, NEO, D], F16)
                    wv = xnp.tile([P, NEO, D], F16)
                    for t, d in ((wk, wk_d), (wq, wq_d), (wv, wv_d)):
                        nc.gpsimd.dma_start(out=t[:], in_=d.rearrange("(eo p) f -> p eo f", p=P))
                    nc.gpsimd.dma_start(out=fc1[:], in_=fc1_d.rearrange("(eo p) f -> p eo f", p=P))
                    nc.gpsimd.dma_start(out=fc2[:], in_=fc2_d.rearrange("(eo p) f -> p eo f", p=P))

                    for g in range(NKT // 4):
                        xq = p1t.tile([P, 4, D], F32, tag="xq")
                        nc.sync.dma_start(
                            out=xq[:],
                            in_=x_d[g * 512:(g + 1) * 512, :].rearrange(
                                "(j p) f -> p j f", p=P))
                        for j in range(4):
                            r = g * 4 + j
                            st6 = p1t.tile([P, 6], F32, tag="st6")
                            nc.vector.bn_stats(out=st6[:], in_=xq[:, j, :])
                            mv = p1t.tile([P, 2], F32, tag="mv")
                            nc.vector.bn_aggr(out=mv[:], in_=st6[:])
                            sig = p1t.tile([P, 1], F32, tag="sig")
                            nc.scalar.activation(out=sig[:], in_=mv[:, 1:2], func=AF.Sqrt,
                                                 bias=eps_t[:], scale=1.0)
                            rstd = p1t.tile([P, 1], F32, tag="rstd")
                            nc.vector.reciprocal_approx_fast(out=rstd[:], in_=sig[:])
                            xh = p1t.tile([P, D], F16, tag="xh")
                            nc.vector.tensor_scalar(out=xh[:], in0=xq[:, j, :],
                                                    scalar1=mv[:, 0:1], scalar2=rstd[:],
                                                    op0=OP.subtract, op1=OP.mult)
                            tp = p1ps.tile([P, NEO, P], F16, tag="tp1")
                            for eo in range(NEO):
                                nc.tensor.transpose(tp[:, eo, :], xh[:, eo * P:(eo + 1) * P], ident[:])
                            nc.scalar.copy(out=xnT[:, :, r * P:(r + 1) * P], in_=tp[:])

                    for mq in range(4):
                        msl = slice(mq * (NKT // 4), (mq + 1) * (NKT // 4))
                        nc.sync.dma_start(out=maskT[:, msl, :], in_=mask_d[:, msl, :])
                    # P2 per node-chunk: KT + V chase the transposes; QT on q-chunks
                    for c in range(NAC):
                        csl = slice(c * 512, (c + 1) * 512)
                        for h in range(HEADS):
                            ps = p2ps.tile([P, 512], F32, tag="ps")
                            for eo in range(NEO):
                                nc.tensor.matmul(ps[:], wk[:, eo, h * HD:(h + 1) * HD],
                                                 xnT[:, eo, csl],
                                                 start=(eo == 0), stop=(eo == NEO - 1))
                            if use_qk_bias:
                                nc.scalar.activation(out=KT[:, h, csl], in_=ps[:],
                                                     func=AF.Copy,
                                                     bias=qkvb[:, NEO + h:NEO + h + 1],
                                                     scale=1.0)
                            else:
                                nc.scalar.copy(out=KT[:, h, csl], in_=ps[:])
                        for kt in range(4 * c, 4 * c + 4):
                            ps = p2ps.tile([P, 512], F32, tag="ps")
                            for eo in range(NEO):
                                nc.tensor.matmul(ps[:], xnT[:, eo, kt * P:(kt + 1) * P],
                                                 wv[:, eo, :],
                                                 start=(eo == 0), stop=(eo == NEO - 1))
                            if use_v_bias:
                                nc.vector.tensor_tensor(out=ps[:], in0=ps[:],
                                                        in1=vb_rep[:], op=OP.add)
                            nc.vector.tensor_copy(out=Vg[:, kt, :], in_=ps[:])
                        if c < NQC:
                            for h in range(HEADS):
                                ps = p2ps.tile([P, 512], F32, tag="ps")
                                for eo in range(NEO):
                                    nc.tensor.matmul(ps[:], wq[:, eo, h * HD:(h + 1) * HD],
                                                     xnT[:, eo, csl],
                                                     start=(eo == 0), stop=(eo == NEO - 1))
                                if use_qk_bias:
                                    nc.scalar.activation(out=QT[:, h, csl], in_=ps[:],
                                                         func=AF.Copy,
                                                         bias=qkvb[:, h:h + 1], scale=1.0)
                                else:
                                    nc.vector.tensor_copy(out=QT[:, h, csl], in_=ps[:])

                # ---- P3 attention (c outer, h inner) + P4 MLP per chunk ----
                with tc.tile_pool(name="expp", bufs=2) as expp, \
                     tc.tile_pool(name="treep", bufs=2) as treep, \
                     tc.tile_pool(name="recp", bufs=2) as recp, \
                     tc.tile_pool(name="sps", bufs=2, space="PSUM") as sps, \
                     tc.tile_pool(name="ops", bufs=1, space="PSUM") as ops, \
                     tc.tile_pool(name="aux", bufs=2, space="PSUM") as aux, \
                     tc.tile_pool(name="mlp", bufs=1) as mlp, \
                     tc.tile_pool(name="p4t", bufs=4) as p4t, \
                     tc.tile_pool(name="fps", bufs=1, space="PSUM") as fps:
                    for c in range(NQC):
                        qsl = slice(c * 512, (c + 1) * 512)
                        for h in range(HEADS):
                            # scores^T -> exp -> mask, per ktile-pair
                            expTm = expp.tile([P, NKT, 512], F16, tag="expTm")
                            for kp in range(NKT // 2):
                                ps = sps.tile([P, 2, 512], F32, tag="sc")
                                for j in range(2):
                                    kt = kp * 2 + j
                                    nc.tensor.matmul(ps[:, j, :],
                                                     KT[:, h, kt * P:(kt + 1) * P],
                                                     QT[:, h, qsl],
                                                     start=True, stop=False)
                                    nc.tensor.matmul(ps[:, j, :], ident[:],
                                                     maskT[:, kt, qsl],
                                                     start=False, stop=True)
                                nc.scalar.activation(
                                    out=expTm[:, kp * 2:kp * 2 + 2, :],
                                    in_=ps[:], func=AF.Exp)
                            # softmax denominators: fp16 adder tree + ones matmul
                            tr = treep.tile([P, NKT // 2, 512], F16, tag="tree")
                            nc.vector.tensor_tensor(out=tr[:], in0=expTm[:, 0:8, :],
                                                    in1=expTm[:, 8:16, :], op=OP.add)
                            nc.vector.tensor_tensor(out=tr[:, 0:4, :], in0=tr[:, 0:4, :],
                                                    in1=tr[:, 4:8, :], op=OP.add)
                            nc.vector.tensor_tensor(out=tr[:, 0:2, :], in0=tr[:, 0:2, :],
                                                    in1=tr[:, 2:4, :], op=OP.add)
                            nc.vector.tensor_tensor(out=tr[:, 0, :], in0=tr[:, 0, :],
                                                    in1=tr[:, 1, :], op=OP.add)
                            pss = aux.tile([P, 512], F32, tag="aux512")
                            nc.tensor.matmul(pss[:], ones_h[:], tr[:, 0, :],
                                             start=True, stop=True)
                            rec = recp.tile([P, 512], F32, tag="rec")
                            nc.vector.reciprocal_approx_fast(out=rec[:], in_=pss[:])
                            # attnV -> out^T, normalize on copyback
                            po = ops.tile([P, 512], F32, tag="attnps")
                            for kt in range(NKT):
                                nc.tensor.matmul(po[:], Vg[:, kt, h * HD:(h + 1) * HD],
                                                 expTm[:, kt, :],
                                                 start=(kt == 0), stop=(kt == NKT - 1))
                            nc.vector.tensor_tensor(out=outT[:, h, qsl], in0=po[:],
                                                    in1=rec[:], op=OP.mult)

                        # ---- P4 for this chunk ----
                        # swish(outT) -> LN1 stats (T layout, ones matmuls)
                        s0T = mlp.tile([P, NEO, 512], F16, tag="s0T")
                        sq1T = mlp.tile([P, NEO, 512], F16, tag="sq1T")
                        th0 = mlp.tile([P, NEO, 512], F16, tag="th0")
                        for eo in range(NEO):
                            nc.scalar.activation(out=th0[:, eo, :], in_=outT[:, eo, qsl],
                                                 func=AF.Tanh, bias=0.0, scale=0.5)
                            nc.vector.tensor_tensor(out=th0[:, eo, :],
                                                    in0=outT[:, eo, qsl],
                                                    in1=th0[:, eo, :], op=OP.mult)
                            nc.vector.tensor_tensor(out=s0T[:, eo, :],
                                                    in0=th0[:, eo, :],
                                                    in1=outT[:, eo, qsl], op=OP.add)
                            nc.vector.tensor_tensor(out=sq1T[:, eo, :],
                                                    in0=s0T[:, eo, :],
                                                    in1=s0T[:, eo, :], op=OP.mult)
                        psm = aux.tile([P, 512], F32, tag="aux512")
                        for eo in range(NEO):
                            nc.tensor.matmul(psm[:], ones_inv[:], s0T[:, eo, :],
                                             start=(eo == 0), stop=(eo == NEO - 1))
                        mu1 = mlp.tile([P, 512], F32, tag="mu1")
                        nc.vector.tensor_copy(out=mu1[:], in_=psm[:])
                        pse = aux.tile([P, 512], F32, tag="aux512")
                        for eo in range(NEO):
                            nc.tensor.matmul(pse[:], ones_inv[:], sq1T[:, eo, :],
                                             start=(eo == 0), stop=(eo == NEO - 1))
                        e1 = mlp.tile([P, 512], F32, tag="e1")
                        nc.vector.tensor_copy(out=e1[:], in_=pse[:])
                        var1 = mlp.tile([P, 512], F32, tag="var1")
                        nc.vector.tensor_tensor(out=var1[:], in0=mu1[:], in1=mu1[:],
                                                op=OP.mult)
                        nc.vector.tensor_tensor(out=var1[:], in0=e1[:], in1=var1[:],
                                                op=OP.subtract)
                        sig1 = mlp.tile([P, 512], F32, tag="sig1")
                        nc.scalar.activation(out=sig1[:], in_=var1[:], func=AF.Sqrt,
                                             bias=eps_t[:], scale=1.0)
                        rstd1 = mlp.tile([P, 512], F32, tag="rstd1")
                        nc.vector.reciprocal_approx_fast(out=rstd1[:], in_=sig1[:])
                        nmu1 = mlp.tile([P, 512], F16, tag="nmu1")
                        nc.vector.tensor_scalar(out=nmu1[:], in0=mu1[:], scalar1=-1.0,
                                                scalar2=None, op0=OP.mult)

                        # fc1 (ln1 standardization folded: K=1 aug + rstd mult + bias via Silu)
                        sig1h = mlp.tile([1, 512], F16, tag="sig1h")
                        nc.vector.tensor_copy(out=sig1h[:], in_=sig1[0:1, :])
                        x2 = mlp.tile([P, NEO, 512], F16, tag="x2")
                        for ft in range(NEO):
                            ps = fps.tile([P, 512], F32, tag="fcps")
                            for eo in range(NEO):
                                nc.tensor.matmul(ps[:], fc1[:, eo, ft * P:(ft + 1) * P],
                                                 s0T[:, eo, :],
                                                 start=(eo == 0), stop=False)
                            nc.tensor.matmul(ps[:], fc1sum[:, ft * P:(ft + 1) * P],
                                             nmu1[0:1, :], start=False, stop=False)
                            nc.tensor.matmul(ps[:], fc1brow[:, ft * P:(ft + 1) * P],
                                             sig1h[:, :], start=False, stop=True)
                            nc.vector.tensor_tensor(out=x2[:, ft, :], in0=ps[:],
                                                    in1=rstd1[:], op=OP.mult)
                        h1sT = mlp.tile([P, NEO, 512], F16, tag="h1sT")
                        th1 = mlp.tile([P, NEO, 512], F16, tag="th1")
                        for ft in range(NEO):
                            nc.scalar.activation(out=th1[:, ft, :], in_=x2[:, ft, :],
                                                 func=AF.Tanh, bias=0.0, scale=0.5)
                            nc.vector.tensor_tensor(out=th1[:, ft, :],
                                                    in0=x2[:, ft, :],
                                                    in1=th1[:, ft, :], op=OP.mult)
                            nc.vector.tensor_tensor(out=h1sT[:, ft, :],
                                                    in0=th1[:, ft, :],
                                                    in1=x2[:, ft, :], op=OP.add)

                        # LN2 stats
                        sqT = mlp.tile([P, NEO, 512], F16, tag="sqT")
                        nc.vector.tensor_tensor(out=sqT[:], in0=h1sT[:], in1=h1sT[:],
                                                op=OP.mult)
                        psm2 = aux.tile([P, 512], F32, tag="aux512")
                        for eo in range(NEO):
                            nc.tensor.matmul(psm2[:], ones_inv[:], h1sT[:, eo, :],
                                             start=(eo == 0), stop=(eo == NEO - 1))
                        mu2 = mlp.tile([P, 512], F32, tag="mu2")
                        nc.vector.tensor_copy(out=mu2[:], in_=psm2[:])
                        pse2 = aux.tile([P, 512], F32, tag="aux512")
                        for eo in range(NEO):
                            nc.tensor.matmul(pse2[:], ones_inv[:], sqT[:, eo, :],
                                             start=(eo == 0), stop=(eo == NEO - 1))
                        e2 = mlp.tile([P, 512], F32, tag="e2")
                        nc.vector.tensor_copy(out=e2[:], in_=pse2[:])
                        var2 = mlp.tile([P, 512], F32, tag="var2")
                        nc.vector.tensor_tensor(out=var2[:], in0=mu2[:], in1=mu2[:],
                                                op=OP.mult)
                        nc.vector.tensor_tensor(out=var2[:], in0=e2[:], in1=var2[:],
                                                op=OP.subtract)
                        sig2 = mlp.tile([1, 512], F16, tag="sig2")
                        nc.scalar.activation(out=sig2[:], in_=var2[0:1, :], func=AF.Sqrt,
                                             bias=eps_t[0:1, :], scale=1.0)
                        nmu2 = mlp.tile([P, 512], F16, tag="nmu2")
                        nc.vector.tensor_scalar(out=nmu2[:], in0=mu2[:], scalar1=-1.0,
                                                scalar2=None, op0=OP.mult)

                        # fc2 in ROW orientation with ln2 standardization via K=1 aug.
                        # rstd2 never applied: psum = h2 * sig2_q > 0 per-row scale,
                        # and the final LayerNorm is invariant to it.
                        for qs in range(4 * c, 4 * c + 4):
                            lsl = slice((qs - 4 * c) * P, (qs - 4 * c + 1) * P)
                            ps = fps.tile([P, 512], F32, tag="fcps")
                            for eo in range(NEO):
                                nc.tensor.matmul(ps[:], h1sT[:, eo, lsl], fc2[:, eo, :],
                                                 start=(eo == 0), stop=False)
                            nc.tensor.matmul(ps[:], nmu2[0:1, lsl], fc2sum[:, :],
                                             start=False, stop=False)
                            nc.tensor.matmul(ps[:], sig2[:, lsl], fc2b[:, :],
                                             start=False, stop=True)
                            st6 = p4t.tile([P, 6], F32, tag="st6b")
                            nc.vector.bn_stats(out=st6[:], in_=ps[:])
                            mv = p4t.tile([P, 2], F32, tag="mvb")
                            nc.vector.bn_aggr(out=mv[:], in_=st6[:])
                            sig = p4t.tile([P, 1], F32, tag="sigb")
                            nc.scalar.activation(out=sig[:], in_=mv[:, 1:2], func=AF.Sqrt,
                                                 bias=eps_t[:], scale=1.0)
                            rstd = p4t.tile([P, 1], F32, tag="rstdb")
                            nc.vector.reciprocal_approx_fast(out=rstd[:], in_=sig[:])
                            yt = p4t.tile([P, D], F32, tag="yt")
                            nc.vector.tensor_scalar(out=yt[:], in0=ps[:],
                                                    scalar1=mv[:, 0:1], scalar2=rstd[:],
                                                    op0=OP.subtract, op1=OP.mult)
                            if use_final_affine:
                                nc.vector.tensor_tensor(out=yt[:], in0=yt[:],
                                                        in1=naff[:, 0:D], op=OP.mult)
                                nc.vector.tensor_tensor(out=yt[:], in0=yt[:],
                                                        in1=naff[:, D:2 * D], op=OP.add)
                            nc.sync.dma_start(out=y_d[qs * P:(qs + 1) * P, :], in_=yt[:])

    nc.compile()
    return nc


def kernel(**inputs):
    x = np.asarray(inputs["x"], np.float32)
    conn = np.asarray(inputs["connectivity"])
    Wq = np.asarray(inputs["Wq"], np.float32)
    Wk = np.asarray(inputs["Wk"], np.float32)
    Wv = np.asarray(inputs["Wv"], np.float32)
    norm_w = np.asarray(inputs["norm_w"], np.float32)
    norm_b = np.asarray(inputs["norm_b"], np.float32)
    ln1_w = np.asarray(inputs["ln1_w"], np.float32)
    ln1_b = np.asarray(inputs["ln1_b"], np.float32)
    fc1_w = np.asarray(inputs["fc1_w"], np.float32)
    fc1_b = np.asarray(inputs["fc1_b"], np.float32)
    ln2_w = np.asarray(inputs["ln2_w"], np.float32)
    ln2_b = np.asarray(inputs["ln2_b"], np.float32)
    fc2_w = np.asarray(inputs["fc2_w"], np.float32)
    fc2_b = np.asarray(inputs["fc2_b"], np.float32)

    s = 1.0 / np.sqrt(HD)
    wq_t = np.ascontiguousarray(norm_w[:, None] * Wq.T)
    wk_t = np.ascontiguousarray((norm_w[:, None] * Wk.T) * np.float32(s))
    wv_t = np.ascontiguousarray(norm_w[:, None] * Wv.T)
    qb = Wq @ norm_b
    kb = (Wk @ norm_b) * s
    vb = Wv @ norm_b
    fc1_t = np.ascontiguousarray(ln1_w[:, None] * fc1_w.T)
    fc1b_eff = fc1_w @ ln1_b + fc1_b
    fc2_t = np.ascontiguousarray(ln2_w[:, None] * fc2_w.T)
    fc2b_eff = fc2_w @ ln2_b + fc2_b
    fc2sum = fc2_t.sum(axis=0)

    use_qk_bias = bool(np.abs(qb).max() > 0 or np.abs(kb).max() > 0)
    use_v_bias = bool(np.abs(vb).max() > 0)
    use_final_affine = not (np.allclose(norm_w, 1.0) and np.allclose(norm_b, 0.0))

    key = (use_v_bias, use_qk_bias, use_final_affine)
    if key not in _compiled:
        _compiled[key] = _build(*key)
    nc = _compiled[key]

    qkvb = np.zeros((P, 3 * NEO), np.float32)
    qkvb[:, 0:NEO] = qb.reshape(NEO, P).T
    qkvb[:, NEO:2 * NEO] = kb.reshape(NEO, P).T
    qkvb[:, 2 * NEO:3 * NEO] = vb.reshape(NEO, P).T
    naff = np.concatenate([norm_w, norm_b]).reshape(1, 2 * D).astype(np.float32)

    common = {
        "wq_t": wq_t.astype(np.float16), "wk_t": wk_t.astype(np.float16),
        "wv_t": wv_t.astype(np.float16),
        "fc1_t": fc1_t.astype(np.float16), "fc2_t": fc2_t.astype(np.float16),
        "fc1b": np.ascontiguousarray(fc1b_eff.reshape(NEO, P).T.astype(np.float32)),
        "fc2sum": fc2sum.reshape(1, D).astype(np.float16),
        "fc2b": fc2b_eff.reshape(1, D).astype(np.float16),
        "fc1sum": fc1_t.sum(axis=0).reshape(1, D).astype(np.float16),
        "fc1brow": fc1b_eff.reshape(1, D).astype(np.float16),
        "qkvb": qkvb, "naff": naff, "vbrow": vb.reshape(1, D).astype(np.float32),
    }

    in_maps = []
    core_ids = list(range(8))
    for c in core_ids:
        b, half = c // 2, c % 2
        qofs = half * QLOC
        xr = np.roll(x[b], -qofs, axis=0)
        cm = np.roll(np.roll(conn[b, 0], -qofs, axis=0), -qofs, axis=1)
        maskT = ((cm[:QLOC, :].T.astype(np.float32) - 1.0) * 30.0).astype(np.float16)  # 0 keep / -30 drop
        maskT = np.ascontiguousarray(
            maskT.reshape(NKT, P, QLOC).transpose(1, 0, 2))          # [P, NKT, QLOC]
        in_maps.append({"x": np.ascontiguousarray(xr), "maskT": maskT, **common})

    res = run_bass_kernel_spmd(nc, in_maps, core_ids)

    y = np.empty((B, A, D), np.float32)
    for c in core_ids:
        b, half = c // 2, c % 2
        y[b, half * QLOC:(half + 1) * QLOC] = res.results[c]["y"]
    return y
